# revision 48
# baseline (speedup 1.0000x reference)
"""CARE-GNN forward on 8 Trainium2 NeuronCores (Bass/Tile).

Strategy (dst-sharded message passing):
  - Nodes are sharded across 8 cores by dst range; each core owns all edges
    into its nodes, so segment sums/counts complete locally (no all-reduce).
  - Per layer, every core reads a full node table (256B rows:
    [h fp8e4m3 x128 | pj' bf16 | pad]) from HBM, rebuilt via two
    contiguous-block AllGathers of per-core slab halves (half-major table
    layout), so the first collective overlaps the producer tail.
  - Edges are laid out slot-aligned: the k-th in-edge of the node at tile
    position p lives at partition p of chunk k. Gathered rows are prescaled
    in place by alpha and accumulated with constant-identity matmuls into
    PSUM; pi[dst] is just the per-tile pi column (no per-edge pi gather).
  - att([h_i, h_j]) = h_i @ wi + h_j @ wj, so alpha needs only per-node
    projections: pi' = h @ wi + att_b stays in SBUF, pj = h @ wj rides in
    the gathered row.
  - dma_gather uses signed int16 indices, so the table is addressed in two
    regions (the two slab-half blocks, both < 32768 rows), sub-gathers of
    <= 11 chunks per call keep SWDGE ring entries small; empty slots gather
    row 0 and are zeroed by a validity mask folded into alpha.
  - Packing: pass 1 fixes each node's slab half by total degree; pass 2
    re-packs within each half by (region-A deg, region-B deg) with 768-block
    re-sorts so both regions' per-tile chunk maxima stay tight (~87% fill).
"""

import os
import sys
import types

import numpy as np
import ml_dtypes

N_CORES = 8
HID = 128
ROW_B = 256  # bytes per table row: [h fp8e4m3 (128B) | pj' bf16 (2B) | pad]
REG_SPLIT_CORE = 5  # table rows of cores 0..4 are region A, 5..7 region B


def _install_axon_ntff_hook():
    """Best-effort shim so trace=True (BASS_TRACE=1) works under axon."""
    try:
        if "antenv.axon_hooks" in sys.modules:
            return
        mod = types.ModuleType("antenv.axon_hooks")
        mod._hook = None
        mod.set_axon_ntff_profile_hook = lambda h: setattr(mod, "_hook", h)
        mod.get_axon_ntff_profile_hook = lambda: mod._hook
        sys.modules["antenv.axon_hooks"] = mod
        import antenv

        antenv.axon_hooks = mod
        from trn_agent_boot.trn_boot import _ntff_profile_via_ctypes

        so = "/opt/axon/libaxon_pjrt.so"
        if os.path.exists(so):
            mod.set_axon_ntff_profile_hook(_ntff_profile_via_ctypes(so))
    except Exception:
        pass


def _host_prep(x, edge_index):
    """Shard nodes/edges, build slot-aligned gather indices. Pure index work."""
    N = x.shape[0]
    src = np.asarray(edge_index[0], dtype=np.int64)
    dst = np.asarray(edge_index[1], dtype=np.int64)
    npc = (N + N_CORES - 1) // N_CORES  # nodes per core
    tpc = (npc + 127) // 128  # tiles per core
    slab = tpc * 128
    H2 = min(tpc - 1, 32767 // (128 * N_CORES)) * 128  # region-A positions per core
    half = N_CORES * H2  # region-A table rows (A block leads the table)

    deg = np.bincount(dst, minlength=N)
    owner = np.minimum(np.arange(N) // npc, N_CORES - 1)

    # pass 1: provisional positions by total degree; fixes each node's half
    # (= its gather region, so the halved AllGather blocks stay contiguous)
    slabrow = np.empty(N, dtype=np.int64)
    for c in range(N_CORES):
        lo, hi = c * npc, min((c + 1) * npc, N)
        o = np.argsort(-deg[lo:hi], kind="stable")
        slabrow[lo + o] = np.arange(hi - lo)
    src_reg = (slabrow >= H2).astype(np.int64)[src]  # edge region
    lowdeg = np.bincount(dst[src_reg == 0], minlength=N)
    highdeg = deg - lowdeg

    # pass 2: repack within each half by (lowdeg, highdeg); 768-block re-sort
    # by highdeg keeps BOTH regions' per-tile chunk maxima tight
    for c in range(N_CORES):
        lo, hi = c * npc, min((c + 1) * npc, N)
        r, l, h = slabrow[lo:hi], lowdeg[lo:hi], highdeg[lo:hi]
        new = np.empty_like(r)
        for p0, p1 in ((0, H2), (H2, slab)):
            m = np.flatnonzero((r >= p0) & (r < p1))
            o = np.lexsort((-h[m], -l[m]))
            oo = np.concatenate(
                [blk[np.argsort(-h[m][blk], kind="stable")] for blk in
                 (o[i : i + 768] for i in range(0, len(o), 768))]
            )
            new[m[oo]] = p0 + np.arange(len(m))
        slabrow[lo:hi] = new

    # table rows: [A block: cores x H2 | B block: cores x (slab - H2)]
    table_row = np.where(
        slabrow < H2,
        owner * H2 + slabrow,
        half + owner * (slab - H2) + (slabrow - H2),
    )

    e_src_row = table_row[src]
    e_core = owner[dst]
    e_slabrow = slabrow[dst]
    e_tile = e_slabrow // 128
    e_pos = e_slabrow % 128
    e_reg = src_reg

    # per-(core,tile,pos,region) sequence number -> chunk index
    key = (((e_core * tpc + e_tile) * 128 + e_pos) * 2 + e_reg).astype(np.int64)
    order = np.argsort(key, kind="stable")
    ks = key[order]
    grp_start = np.r_[0, np.flatnonzero(np.diff(ks)) + 1]
    grp_len = np.diff(np.r_[grp_start, len(ks)])
    seq = np.arange(len(ks)) - np.repeat(grp_start, grp_len)
    e_seq = np.empty(len(ks), dtype=np.int64)
    e_seq[order] = seq

    # chunk counts per (core, tile, region) = max over positions of count
    cnt = np.zeros((N_CORES, tpc, 128, 2), dtype=np.int64)
    np.add.at(cnt, (e_core, e_tile, e_pos, e_reg), 1)
    c_reg = cnt.max(axis=2).max(axis=0)  # [tpc, 2] shared across cores (SPMD)
    CA = c_reg[:, 0].astype(int)
    CB = c_reg[:, 1].astype(int)
    CT = CA + CB
    assert CT.min() >= 1

    offs_a = np.r_[0, np.cumsum(CA)].astype(int)
    offs_b = np.r_[0, np.cumsum(CB)].astype(int)
    offs_t = np.r_[0, np.cumsum(CT)].astype(int)
    tot_a, tot_b, tot_t = int(offs_a[-1]), int(offs_b[-1]), int(offs_t[-1])

    # gather index arrays (slot i within a region block = chunk*128 + pos)
    idx_a = np.zeros((N_CORES, max(tot_a, 1) * 128), dtype=np.int64)
    idx_b = np.zeros((N_CORES, max(tot_b, 1) * 128), dtype=np.int64)
    mask = np.zeros((N_CORES, 128, tot_t), dtype=np.float32)

    e_off = np.where(e_reg == 0, offs_a[e_tile] * 128, offs_b[e_tile] * 128)
    e_lin = e_off + e_seq * 128 + e_pos
    e_val = np.where(e_reg == 0, e_src_row, e_src_row - half)
    for c in range(N_CORES):
        m = e_core == c
        ra = m & (e_reg == 0)
        rb = m & (e_reg == 1)
        idx_a[c, e_lin[ra]] = e_val[ra]
        idx_b[c, e_lin[rb]] = e_val[rb]
        # mask columns: tile t occupies [offs_t[t], offs_t[t+1]) = [A.. | B..]
        mask[c, e_pos[ra], offs_t[e_tile[ra]] + e_seq[ra]] = 1.0
        mask[c, e_pos[rb], offs_t[e_tile[rb]] + CA[e_tile[rb]] + e_seq[rb]] = 1.0

    def wrap16(lin):  # [n] -> [128, n//16] int16 (16-part wrap, replicated x8)
        w = lin.reshape(-1, 16).T.astype(np.uint16).view(np.int16)  # [16, n/16]
        return np.tile(w, (8, 1))

    idx_a16 = np.stack([wrap16(idx_a[c]) for c in range(N_CORES)])
    idx_b16 = np.stack([wrap16(idx_b[c]) for c in range(N_CORES)])

    # inverse degree, laid out [pos, tile]; holes -> 1.0
    invdeg = np.ones((N_CORES, slab), dtype=np.float32)
    invdeg[owner, slabrow] = 1.0 / np.maximum(deg, 1).astype(np.float32)
    invdeg = invdeg.reshape(N_CORES, tpc, 128).transpose(0, 2, 1).copy()

    # x slabs, transposed: [in_dim, slab] per core (bf16 for cheap loads)
    in_dim = x.shape[1]
    xT = np.zeros((N_CORES, in_dim, slab), dtype=ml_dtypes.bfloat16)
    for c in range(N_CORES):
        lo, hi = c * npc, min((c + 1) * npc, N)
        xT[c][:, slabrow[lo:hi]] = (
            np.asarray(x[lo:hi], dtype=np.float32).T.astype(ml_dtypes.bfloat16)
        )

    return dict(
        N=N, npc=npc, tpc=tpc, slab=slab, half=half, H2=H2, in_dim=in_dim,
        CA=CA, CB=CB, offs_a=offs_a, offs_b=offs_b, offs_t=offs_t,
        tot_a=tot_a, tot_b=tot_b, tot_t=tot_t,
        idx_a16=idx_a16, idx_b16=idx_b16, mask=mask, invdeg=invdeg, xT=xT,
        owner=owner, slabrow=slabrow,
    )


def _build_program(p, consts):
    import concourse.bacc as bacc
    import concourse.mybir as mybir
    import concourse.tile as tile

    f32 = mybir.dt.float32
    bf16 = mybir.dt.bfloat16
    fp8 = mybir.dt.float8e4
    u8 = mybir.dt.uint8
    i16 = mybir.dt.int16
    AF = mybir.ActivationFunctionType

    tpc, slab, in_dim, half = p["tpc"], p["slab"], p["in_dim"], p["half"]
    CA, CB = p["CA"], p["CB"]
    offs_a, offs_b, offs_t = p["offs_a"], p["offs_b"], p["offs_t"]
    tot_a, tot_b, tot_t = p["tot_a"], p["tot_b"], p["tot_t"]
    att_bs = (consts["att1_b"], consts["att2_b"])
    gtab = N_CORES * slab
    nk = in_dim // 128  # contraction tiles for the encoder

    nc = bacc.Bacc("TRN2", num_devices=N_CORES, num_swdge_queues=4, dynamic_dma_scratch_size=49152)

    # ---- I/O ----
    xT = nc.dram_tensor("xT", [in_dim, slab], bf16, kind="ExternalInput")
    idxA = nc.dram_tensor("idxA", [128, max(tot_a, 1) * 8], i16, kind="ExternalInput")
    idxB = nc.dram_tensor("idxB", [128, max(tot_b, 1) * 8], i16, kind="ExternalInput")
    maskT = nc.dram_tensor("maskT", [128, tot_t], bf16, kind="ExternalInput")
    invdeg = nc.dram_tensor("invdeg", [128, tpc], f32, kind="ExternalInput")
    encw = nc.dram_tensor("encw", [in_dim, HID], bf16, kind="ExternalInput")
    encb = nc.dram_tensor("encb", [HID, 1], f32, kind="ExternalInput")
    w4 = nc.dram_tensor("w4", [HID, 4], f32, kind="ExternalInput")
    clsw = nc.dram_tensor("clsw", [HID, 2], f32, kind="ExternalInput")
    clsb = nc.dram_tensor("clsb", [1, 2], f32, kind="ExternalInput")
    ident_in = nc.dram_tensor("ident", [128, 128], f32, kind="ExternalInput")
    # logits packed [pos, tile*2]; host unshards via (slabrow%128, slabrow//128)
    logits = nc.dram_tensor("logits", [128, tpc * 2], f32, kind="ExternalOutput")

    # ---- internal DRAM ----
    slabs = [nc.dram_tensor(f"slab{l}", [slab, ROW_B], u8) for l in (1, 2)]
    tables = [
        nc.dram_tensor(f"table{l}", [gtab, ROW_B], u8, addr_space="Shared")
        for l in (1, 2)
    ]

    with tile.TileContext(nc) as tc:
        with (
            tc.tile_pool(name="const", bufs=1) as cpool,
            tc.tile_pool(name="work", bufs=3) as pool,
            tc.tile_pool(name="gath", bufs=4) as gpool,
            tc.tile_pool(name="psacc", bufs=3, space="PSUM") as ps_acc,
            tc.tile_pool(name="pstr", bufs=3, space="PSUM") as ps_tr,
            tc.tile_pool(name="pssm", bufs=2, space="PSUM") as ps_sm,
        ):
            # ---- constants / whole-kernel residents in SBUF ----
            encw_t = [cpool.tile([128, HID], bf16, tag=f"encw{i}", name=f"encw{i}") for i in range(nk)]
            for i, t in enumerate(encw_t):
                nc.sync.dma_start(out=t[:], in_=encw[i * 128 : (i + 1) * 128, :])
            encb_t = cpool.tile([HID, 1], f32, tag="encb")
            nc.sync.dma_start(out=encb_t[:], in_=encb[:])
            w4_t = cpool.tile([HID, 4], f32, tag="w4")
            nc.sync.dma_start(out=w4_t[:], in_=w4[:])
            clsw_t = cpool.tile([HID, 2], f32, tag="clsw")
            nc.sync.dma_start(out=clsw_t[:], in_=clsw[:])
            clsb_t = cpool.tile([1, 2], f32, tag="clsb")
            nc.sync.dma_start(out=clsb_t[:], in_=clsb[:])
            ident_f = cpool.tile([128, 128], f32, tag="identf")
            nc.sync.dma_start(out=ident_f[:], in_=ident_in[:])
            ident_b = cpool.tile([128, 128], bf16, tag="identb")
            nc.vector.tensor_copy(out=ident_b[:], in_=ident_f[:])
            ones_f = cpool.tile([1, 128], f32, tag="onesf")
            nc.vector.memset(ones_f[:], 1.0)
            inv_all = cpool.tile([128, tpc], f32, tag="invall")
            nc.sync.dma_start(out=inv_all[:], in_=invdeg[:])
            idxA_t = cpool.tile([128, max(tot_a, 1) * 8], i16, tag="idxAt")
            nc.sync.dma_start(out=idxA_t[:], in_=idxA[:])
            idxB_t = cpool.tile([128, max(tot_b, 1) * 8], i16, tag="idxBt")
            nc.sync.dma_start(out=idxB_t[:], in_=idxB[:])
            mask_all = cpool.tile([128, tot_t], bf16, tag="maskall")
            nc.sync.dma_start(out=mask_all[:], in_=maskT[:])
            pi_all = [
                cpool.tile([128, tpc], f32, tag=f"piall{l}", name=f"piall{l}")
                for l in (1, 2)
            ]
            lg_all = cpool.tile([128, tpc * 2], f32, tag="lgall")

            def p_phase_and_store(hT_sb, t, layer):
                """hT (f32 [hid, nodes]) -> slab rows [h|pj'] + pi' column."""
                co = t * 128
                lw = 2 * (layer - 1)
                h_ps = ps_tr.tile([128, 128], f32, tag="tr")
                nc.tensor.transpose(out=h_ps[:], in_=hT_sb[:], identity=ident_f[:])
                hp = pool.tile([128, 130], u8, tag="hp")
                nc.vector.tensor_copy(out=hp[:, 0:128].bitcast(fp8), in_=h_ps[:])
                p_ps = ps_sm.tile([128, 2], f32, tag="sm")
                nc.tensor.matmul(
                    out=p_ps[:], lhsT=hT_sb[:], rhs=w4_t[:, lw : lw + 2],
                    start=True, stop=True,
                )
                # pi' = pi + att_b kept in SBUF for the sigmoid bias
                nc.scalar.add(
                    out=pi_all[layer - 1][:, t : t + 1],
                    in_=p_ps[:, 0:1],
                    add=float(att_bs[layer - 1]),
                )
                nc.vector.tensor_copy(
                    out=hp[:, 128:130].bitcast(bf16), in_=p_ps[:, 1:2]
                )
                nc.sync.dma_start(
                    out=slabs[layer - 1][co : co + 128, 0:130], in_=hp[:]
                )

            # ---- encoder: hT = relu(encw.T @ xT + encb), then p1 ----
            for t in range(tpc):
                co = t * 128
                xt = [pool.tile([128, 128], bf16, tag=f"xt{i}", name=f"xt{i}") for i in range(nk)]
                for i, xx in enumerate(xt):
                    nc.sync.dma_start(
                        out=xx[:], in_=xT[i * 128 : (i + 1) * 128, co : co + 128]
                    )
                hT_ps = ps_tr.tile([128, 128], f32, tag="tr")
                for i in range(nk):
                    nc.tensor.matmul(
                        out=hT_ps[:], lhsT=encw_t[i][:], rhs=xt[i][:],
                        start=(i == 0), stop=(i == nk - 1),
                    )
                hT_sb = pool.tile([128, 128], f32, tag="hTsb")
                nc.scalar.activation(
                    out=hT_sb[:], in_=hT_ps[:], func=AF.Relu, bias=encb_t[:]
                )
                p_phase_and_store(hT_sb, t, layer=1)

            # ---- two message-passing layers ----
            gather_ctr = [0]
            H2 = p["H2"]

            def emit_ag(layer, piece):
                """AllGather one slab half into the half-major table block."""
                (h0, h1), (r0, r1) = (
                    ((0, H2), (0, half)),
                    ((H2, slab), (half, gtab)),
                )[piece]
                nc.gpsimd.collective_compute(
                    "AllGather",
                    mybir.AluOpType.bypass,
                    replica_groups=[list(range(N_CORES))],
                    ins=[slabs[layer - 1][h0:h1, :]],
                    outs=[tables[layer - 1][r0:r1, :]],
                )

            for layer in (1, 2):
                table = tables[layer - 1]
                # two AllGathers into the half-major table: the big piece
                # (region A, 31/49 tiles) fires as soon as those tiles stored
                # their rows, overlapping the producer tail; outs contiguous
                emit_ag(layer, 0)
                emit_ag(layer, 1)
                for t in range(tpc):
                    co = t * 128
                    ca, cb = int(CA[t]), int(CB[t])
                    ct = ca + cb
                    g = []
                    for reg, cr, idx_t, offs, reg_ap in (
                        (0, ca, idxA_t, offs_a, table[0:half, :]),
                        (1, cb, idxB_t, offs_b, table[half:gtab, :]),
                    ):
                        if cr == 0:
                            g.append(None)
                            continue
                        gt = gpool.tile([128, cr, ROW_B], u8, tag=f"g{reg}", name=f"g{reg}")
                        # sub-gathers keep ring-buffer entries small so several
                        # stay in flight and the DMA engines never starve
                        SPLIT = 11
                        for s0 in range(0, cr, SPLIT):
                            sc = min(SPLIT, cr - s0)
                            nc.gpsimd.dma_gather(
                                out_ap=gt[:, s0 : s0 + sc, :],
                                in_ap=reg_ap,
                                idxs_ap=idx_t[:, (offs[t] + s0) * 8 : (offs[t] + s0 + sc) * 8],
                                num_idxs=sc * 128,
                                num_idxs_reg=sc * 128,
                                elem_size=ROW_B,
                                single_packet=False,
                                queue_num=gather_ctr[0] % 4,
                            )
                            gather_ctr[0] += 1
                        g.append(gt)
                    # alpha = sigmoid(pj + pi') * mask  (bf16 [128, ct])
                    alpha = pool.tile([128, ct], bf16, tag="alpha")
                    if ca:
                        nc.scalar.activation(
                            out=alpha[:, 0:ca, None],
                            in_=g[0][:, :, 128:130].bitcast(bf16),
                            func=AF.Sigmoid,
                            bias=pi_all[layer - 1][:, t : t + 1],
                        )
                    if cb:
                        nc.scalar.activation(
                            out=alpha[:, ca:ct, None],
                            in_=g[1][:, :, 128:130].bitcast(bf16),
                            func=AF.Sigmoid,
                            bias=pi_all[layer - 1][:, t : t + 1],
                        )
                    nc.vector.tensor_tensor(
                        out=alpha[:], in0=alpha[:],
                        in1=mask_all[:, offs_t[t] : offs_t[t] + ct],
                        op=mybir.AluOpType.mult,
                    )
                    # scaled = fp8 h rows * alpha  (bf16 for the matmul rhs)
                    sc_t = []
                    for reg, cr, sl in ((0, ca, slice(0, ca)), (1, cb, slice(ca, ct))):
                        if cr == 0:
                            sc_t.append(None)
                            continue
                        st = pool.tile(
                            [128, cr, HID], bf16, tag=f"sc{reg}", name=f"sc{reg}", bufs=3
                        )
                        nc.vector.tensor_tensor(
                            out=st[:],
                            in0=g[reg][:, :, 0:128].bitcast(fp8),
                            in1=alpha[:, sl, None].to_broadcast([128, cr, HID]),
                            op=mybir.AluOpType.mult,
                        )
                        sc_t.append(st)
                    # acc = sum_k I @ scaled_k  (PSUM accumulation, const weights)
                    acc = ps_acc.tile([128, HID], f32, tag="acc")
                    for k in range(ct):
                        reg, c = (0, k) if k < ca else (1, k - ca)
                        nc.tensor.matmul(
                            out=acc[:], lhsT=ident_b[:], rhs=sc_t[reg][:, c, :],
                            start=(k == 0), stop=(k == ct - 1),
                        )
                    inv_col = inv_all[:, t : t + 1]
                    if layer == 1:
                        # h2 = relu(acc * invdeg); p2 phase + stores
                        h2_sb = pool.tile([128, 128], f32, tag="h2sb")
                        nc.scalar.activation(
                            out=h2_sb[:], in_=acc[:], func=AF.Relu, scale=inv_col
                        )
                        hT2_ps = ps_tr.tile([128, 128], f32, tag="tr")
                        nc.tensor.transpose(
                            out=hT2_ps[:], in_=h2_sb[:], identity=ident_f[:]
                        )
                        hT2_sb = pool.tile([128, 128], f32, tag="hT2sb")
                        nc.vector.tensor_copy(out=hT2_sb[:], in_=hT2_ps[:])
                        p_phase_and_store(hT2_sb, t, layer=2)
                    else:
                        # logits = (acc * invdeg) @ clsw + clsb
                        m_sb = pool.tile([128, 128], f32, tag="msb")
                        nc.scalar.mul(out=m_sb[:], in_=acc[:], mul=inv_col)
                        mT_ps = ps_tr.tile([128, 128], f32, tag="tr")
                        nc.tensor.transpose(
                            out=mT_ps[:], in_=m_sb[:], identity=ident_f[:]
                        )
                        mT_sb = pool.tile([128, 128], f32, tag="mTsb")
                        nc.vector.tensor_copy(out=mT_sb[:], in_=mT_ps[:])
                        lg_ps = ps_sm.tile([128, 2], f32, tag="sm")
                        nc.tensor.matmul(
                            out=lg_ps[:], lhsT=mT_sb[:], rhs=clsw_t[:],
                            start=True, stop=False,
                        )
                        nc.tensor.matmul(
                            out=lg_ps[:], lhsT=ones_f[:], rhs=clsb_t[:],
                            start=False, stop=True,
                        )
                        nc.vector.tensor_copy(
                            out=lg_all[:, 2 * t : 2 * t + 2], in_=lg_ps[:]
                        )
            nc.sync.dma_start(out=logits[:], in_=lg_all[:])

    nc.compile()
    return nc


_CACHE = {}


def kernel(**inputs):
    _install_axon_ntff_hook()
    from concourse import bass_utils

    bass_utils.upload_artifacts = lambda tmpdir: tmpdir

    x = np.asarray(inputs["x"], dtype=np.float32)
    edge_index = np.asarray(inputs["edge_index"])
    p = _host_prep(x, edge_index)

    consts = dict(
        att1_b=float(np.asarray(inputs["att1_b"]).reshape(-1)[0]),
        att2_b=float(np.asarray(inputs["att2_b"]).reshape(-1)[0]),
    )
    key = (tuple(p["CA"]), tuple(p["CB"]), consts["att1_b"], consts["att2_b"])
    if key not in _CACHE:
        _CACHE[key] = _build_program(p, consts)
    nc = _CACHE[key]

    w4 = np.concatenate(
        [
            np.asarray(inputs["att1_w"], dtype=np.float32).reshape(2, HID).T,
            np.asarray(inputs["att2_w"], dtype=np.float32).reshape(2, HID).T,
        ],
        axis=1,
    )  # [HID, 4] = [wi1, wj1, wi2, wj2]
    common = dict(
        encw=np.ascontiguousarray(
            np.asarray(inputs["enc_w"], dtype=np.float32).astype(ml_dtypes.bfloat16)
        ),
        encb=np.asarray(inputs["enc_b"], dtype=np.float32).reshape(HID, 1),
        w4=np.ascontiguousarray(w4),
        clsw=np.ascontiguousarray(np.asarray(inputs["cls_w"], dtype=np.float32)),
        clsb=np.asarray(inputs["cls_b"], dtype=np.float32).reshape(1, 2),
        ident=np.eye(128, dtype=np.float32),
    )
    in_maps = []
    for c in range(N_CORES):
        in_maps.append(
            dict(
                xT=np.ascontiguousarray(p["xT"][c]),
                idxA=np.ascontiguousarray(p["idx_a16"][c]),
                idxB=np.ascontiguousarray(p["idx_b16"][c]),
                maskT=np.ascontiguousarray(p["mask"][c].astype(ml_dtypes.bfloat16)),
                invdeg=np.ascontiguousarray(p["invdeg"][c]),
                **common,
            )
        )

    res = bass_utils.run_bass_kernel_spmd(nc, in_maps, core_ids=list(range(N_CORES)))
    kernel.last_result = res

    N = p["N"]
    tpc = p["tpc"]
    out = np.zeros((N, 2), dtype=np.float32)
    for c in range(N_CORES):
        m = p["owner"] == c
        arr = np.asarray(res.results[c]["logits"], dtype=np.float32)
        arr = arr.reshape(128, tpc, 2).transpose(1, 0, 2).reshape(-1, 2)
        out[m] = arr[p["slabrow"][m]]
    return out

